# revision 49
# baseline (speedup 1.0000x reference)
"""Causal multi-head self-attention with RoPE on 8 TRN2 NeuronCores.

Problem: b=4, s=2048, d_model=1024, 16 heads, d_k=64, fp32 I/O.

Sharding: core c = (batch b = c//2, head-half g = c%2). Each core computes the
8 heads of one head-half for one batch element, applies its slice of the
output projection, and returns a partial [2048, 1024] (bf16); the host sums
the two partials per batch in fp32 (the tensor-parallel all-reduce on host).

v3 schedule (v2 + pipeline/latency work):
- exp is split into per-head-half activations (and per-half diag masks +
  per-half PSUM drain copies) so each AV matmul waits ~600ns of scalar
  latency instead of ~1us, cutting per-step PE bubbles.
- the output projection is emitted per seq-group as filler work inside the
  LAST pair's attention stream (gated on that pair's normalization), so the
  former ~36us serial o_proj tail shrinks to one seq-group (~10us).
- pair-0 Q/K/V projection units go through the need-key queue like everyone
  else, so attention(0, gq0) starts after 2 qk units + 4 v units instead of
  after all 16; the input DMA order matches this consumption order.
- the two denominator-broadcast matmuls are column-tiled to disjoint output
  partitions (A->0:64, B->64:128) and run concurrently in the PE array, with
  one merged reciprocal.

Score matmul halves (contraction 64, partitions 0-63/64-127) auto-derive
tile_position (0,0)/(64,0) and stream concurrently in the PE array.

PSUM banking (8 banks of 2KB/partition): avAB accumulator tag "av" 1x2 banks,
score tiles tag "u" 2x2 banks (pipeline depth 2), projection tiles tag "pj"
2x1 banks.
"""

import collections
import numpy as np
import ml_dtypes

bf16 = ml_dtypes.bfloat16

N_HEADS = 16
THETA = 10000.0
B, S, D = 4, 2048, 1024
DK = D // N_HEADS          # 64
DH = D // 2                # 512 dims per core (8 heads)
P = 128
NKC = D // P               # 8 contraction chunks for projections
NSG = S // 512             # 4 seq groups of 512
NST = S // P               # 16 seq tiles of 128
NPAIR = DH // P            # 4 head-pair tiles per core
VW = 66                    # per-head stride in interleaved V tile

_CACHE = {}


def _build_program():
    import concourse.tile as tile
    from concourse import bacc, mybir

    nc = bacc.Bacc("TRN2", target_bir_lowering=False, debug=False, num_devices=1)
    dt = mybir.dt

    xt_d = nc.dram_tensor("xt", [D, S], dt.bfloat16, kind="ExternalInput")
    wq_d = nc.dram_tensor("wq", [D, DH], dt.bfloat16, kind="ExternalInput")
    wk_d = nc.dram_tensor("wk", [D, DH], dt.bfloat16, kind="ExternalInput")
    wv_d = nc.dram_tensor("wv", [D, DH], dt.bfloat16, kind="ExternalInput")
    wo_d = nc.dram_tensor("wo", [DH, D], dt.bfloat16, kind="ExternalInput")
    cos_d = nc.dram_tensor("cosE", [P, S], dt.bfloat16, kind="ExternalInput")
    sin_d = nc.dram_tensor("sinE", [P, S], dt.bfloat16, kind="ExternalInput")
    tri_d = nc.dram_tensor("tri", [P, P], dt.bfloat16, kind="ExternalInput")
    bc_d = nc.dram_tensor("bcsel", [P, 256], dt.bfloat16, kind="ExternalInput")
    out_d = nc.dram_tensor("out", [S, D], dt.bfloat16, kind="ExternalOutput")

    EXP = mybir.ActivationFunctionType.Exp

    with tile.TileContext(nc) as tc:
        with tc.tile_pool(name="const", bufs=1) as cst, \
             tc.tile_pool(name="persist", bufs=1) as per, \
             tc.tile_pool(name="work", bufs=4) as wkp, \
             tc.tile_pool(name="ev", bufs=2) as evp, \
             tc.tile_pool(name="up", bufs=2, space="PSUM") as pup:

            # ---- input DMA: rotate across 3 engine queues, ordered to match
            # compute consumption: (wq,xt_sg0,wk) + p2/cos/sin sg0 first, then
            # wv (for v units 0-3), then the later seq groups, bcsel, wo last.
            qs_ = [nc.sync, nc.scalar, nc.gpsimd]
            dma_i = [0]

            def dma_load(dst, src):
                qs_[dma_i[0] % 3].dma_start(dst, src)
                dma_i[0] += 1

            wq, wk_, wv, wo = [], [], [], []
            xt = [[None] * NSG for _ in range(NKC)]
            cosE = [None] * NSG
            sinE = [None] * NSG

            def _load_cs(sg):
                c = cst.tile([P, 512], dt.bfloat16, tag=f"cos{sg}", name=f"cos{sg}")
                dma_load(c[:], cos_d.ap()[:, 512 * sg:512 * (sg + 1)])
                cosE[sg] = c
                s = cst.tile([P, 512], dt.bfloat16, tag=f"sin{sg}", name=f"sin{sg}")
                dma_load(s[:], sin_d.ap()[:, 512 * sg:512 * (sg + 1)])
                sinE[sg] = s

            def _load_xt(kc, sg):
                t = cst.tile([P, 512], dt.bfloat16, tag=f"xt{kc}_{sg}",
                             name=f"xt{kc}_{sg}")
                dma_load(t[:], xt_d.ap()[P * kc:P * (kc + 1),
                                         512 * sg:512 * (sg + 1)])
                xt[kc][sg] = t

            for kc in range(NKC):
                t = cst.tile([P, DH], dt.bfloat16, tag=f"wq{kc}", name=f"wq{kc}")
                dma_load(t[:], wq_d.ap()[P * kc:P * (kc + 1), :])
                wq.append(t)
                _load_xt(kc, 0)
                t = cst.tile([P, DH], dt.bfloat16, tag=f"wk{kc}", name=f"wk{kc}")
                dma_load(t[:], wk_d.ap()[P * kc:P * (kc + 1), :])
                wk_.append(t)
                if kc == 1:
                    tri = cst.tile([P, P], dt.bfloat16, tag="tri")
                    dma_load(tri[:], tri_d.ap())
                if kc == 3:
                    _load_cs(0)
            for kc in range(NKC):
                t = cst.tile([P, DH], dt.bfloat16, tag=f"wv{kc}", name=f"wv{kc}")
                dma_load(t[:], wv_d.ap()[P * kc:P * (kc + 1), :])
                wv.append(t)
                _load_xt(kc, 1)
                if kc == 0:
                    _load_cs(1)
            bcsel = cst.tile([P, 256], dt.bfloat16, tag="bc")
            for kc in range(NKC):
                _load_xt(kc, 2)
                if kc == 0:
                    _load_cs(2)
                if kc == 4:
                    dma_load(bcsel[:], bc_d.ap())
            for kc in range(NKC):
                _load_xt(kc, 3)
                if kc == 0:
                    _load_cs(3)
            for kc in range(NPAIR):
                t = cst.tile([P, D], dt.bfloat16, tag=f"wo{kc}", name=f"wo{kc}")
                dma_load(t[:], wo_d.ap()[P * kc:P * (kc + 1), :])
                wo.append(t)

            qrot = [per.tile([P, S], dt.bfloat16, tag=f"qrot{t_i}",
                             name=f"qrot{t_i}") for t_i in range(NPAIR)]
            krot = [per.tile([P, S], dt.bfloat16, tag=f"krot{t_i}",
                             name=f"krot{t_i}") for t_i in range(NPAIR)]
            vil = [per.tile([P, 8 * VW], dt.bfloat16, tag=f"v{m}",
                            name=f"vil{m}") for m in range(NST)]
            conc = [per.tile([P, S], dt.bfloat16, tag=f"conc{t_i}",
                             name=f"conc{t_i}") for t_i in range(NPAIR)]
            rcA = evp.tile([64, S], dt.float32, tag="rcA", name="rcA", bufs=1)
            rcB = evp.tile([64, S], dt.float32, tag="rcB", name="rcB", bufs=1)
            evs = [None] * NPAIR

            # ---- output DMA: keep the busy scalar queue (exp) clear; split
            # each [128,512] store in half across distinct queues so the
            # final store drains ~2x faster (per-queue DMA bw is the limit).
            st_i = [0]

            def dma_store(dst, src):
                qs = (nc.sync, nc.scalar, nc.gpsimd)
                qs[st_i[0] % 3].dma_start(dst[:, 0:256], src[:, 0:256])
                qs[(st_i[0] + 1) % 3].dma_start(dst[:, 256:512], src[:, 256:512])
                st_i[0] += 2

            # ---- work-unit generators; each yield = one PE matmul emitted
            # "u" borrows the (idle-at-startup) score-tile banks so four
            # projection accumulators can be live at once during the
            # DMA-paced startup round-robin
            ps_i = [0]

            def alloc_ps(ptag=None):
                if ptag is None:
                    ptag = ("pj", "u")[ps_i[0] % 2]
                    ps_i[0] += 1
                if ptag == "u":
                    t = pup.tile([P, 1024], dt.float32, tag="u", bufs=2,
                                 name="sAB")
                    return t[:, 0:512]
                return pup.tile([P, 512], dt.float32, tag="pj", bufs=2,
                                name="pjps")[:]

            # rope pair partner: dims are host-relabeled rotate-half-in-32
            # (x1 at block pos 0-15, x2 at 16-31) so the pair swap is a DVE
            # stream_shuffle instead of a PE permutation matmul; the sign
            # (-x2, +x1) is folded into the host sinE table.
            SWAP_MASK = list(range(16, 32)) + list(range(16))

            def qk_unit(t_i, sg, which):
                w_tiles, rot = ((wq, qrot), (wk_, krot))[which]
                ps = alloc_ps()
                for kc in range(NKC):
                    nc.tensor.matmul(ps, w_tiles[kc][:, P * t_i:P * (t_i + 1)],
                                     xt[kc][sg][:],
                                     start=(kc == 0), stop=(kc == NKC - 1))
                    yield
                qsb = wkp.tile([P, 512], dt.bfloat16, tag="qsb")
                nc.vector.tensor_copy(qsb[:], ps)
                gs = slice(512 * sg, 512 * (sg + 1))
                tmp1 = wkp.tile([P, 512], dt.bfloat16, tag="tmp1")
                nc.vector.tensor_mul(tmp1[:], qsb[:], cosE[sg][:])
                q2sb = wkp.tile([P, 512], dt.bfloat16, tag="q2sb")
                nc.vector.stream_shuffle(q2sb[:], qsb[:], SWAP_MASK)
                tmp2 = wkp.tile([P, 512], dt.bfloat16, tag="tmp2")
                nc.vector.tensor_mul(tmp2[:], q2sb[:], sinE[sg][:])
                nc.vector.tensor_add(rot[t_i][:, gs], tmp1[:], tmp2[:])

            def v_unit(m):
                ps = alloc_ps()
                sg, mo = divmod(m, 4)
                for kc in range(NKC):
                    nc.tensor.matmul(ps, xt[kc][sg][:, P * mo:P * (mo + 1)],
                                     wv[kc][:, :],
                                     start=(kc == 0), stop=(kc == NKC - 1))
                    yield
                v3 = vil[m][:].rearrange("p (h c) -> p h c", c=VW)
                nc.gpsimd.memset(v3[:, :, 64:65], 1.0)
                nc.vector.tensor_copy(v3[:, :, 0:64],
                                      ps.rearrange("p (h c) -> p h c", c=64))

            def bc_unit(t_i, gq):
                # broadcast the denominator row to 64 partitions + reciprocal
                ev = evs[t_i]
                gs = slice(512 * gq, 512 * (gq + 1))
                bcA = alloc_ps()
                nc.tensor.matmul(bcA[0:64, :], bcsel[64:65, 0:64],
                                 ev[64:65, 1024 * gq:1024 * gq + 512],
                                 start=True, stop=True)
                yield
                nc.vector.reciprocal_approx_fast(rcA[:, gs], bcA[0:64, :])
                bcB = alloc_ps()
                nc.tensor.matmul(bcB[0:64, :], bcsel[64:65, 0:64],
                                 ev[64:65, 1024 * gq + 512:1024 * (gq + 1)],
                                 start=True, stop=True)
                yield
                nc.vector.reciprocal_approx_fast(rcB[:, gs], bcB[0:64, :])

            def norm_unit(t_i, gq):
                ev = evs[t_i]
                gs = slice(512 * gq, 512 * (gq + 1))
                nc.gpsimd.tensor_mul(conc[t_i][0:64, gs],
                                     ev[0:64, 1024 * gq:1024 * gq + 512],
                                     rcA[:, gs])
                yield
                nc.gpsimd.tensor_mul(conc[t_i][64:128, gs],
                                     ev[0:64, 1024 * gq + 512:1024 * (gq + 1)],
                                     rcB[:, gs])
                yield

            def oproj_unit(gq):
                # output projection for this seq-group's 4 m-tiles; emitted
                # as filler inside the last pair's attention stream
                for m in range(4 * gq, 4 * (gq + 1)):
                    for ng in range(2):
                        ps = alloc_ps()
                        for t_i in range(NPAIR):
                            nc.tensor.matmul(ps[:],
                                             conc[t_i][:, P * m:P * (m + 1)],
                                             wo[t_i][:, 512 * ng:512 * (ng + 1)],
                                             start=(t_i == 0),
                                             stop=(t_i == NPAIR - 1))
                            yield
                        osb = wkp.tile([P, 512], dt.bfloat16, tag="osb")
                        nc.vector.tensor_copy(osb[:], ps[:])
                        dma_store(out_d.ap()[P * m:P * (m + 1),
                                             512 * ng:512 * (ng + 1)], osb[:])

            # the LAST seq-group of the last pair would otherwise leave 32
            # serial matmuls after the final normalization; precompute the
            # pairs-0..2 partial sums as fillers, leaving an 8-matmul tail.
            opart = [per.tile([P, 512], dt.bfloat16, tag=f"opart{i}",
                              name=f"opart{i}") for i in range(8)]

            def oproj_partial_unit(gq):
                for i, m in enumerate(range(4 * gq, 4 * (gq + 1))):
                    for ng in range(2):
                        ps = alloc_ps()
                        for t_i in range(NPAIR - 1):
                            nc.tensor.matmul(ps[:],
                                             conc[t_i][:, P * m:P * (m + 1)],
                                             wo[t_i][:, 512 * ng:512 * (ng + 1)],
                                             start=(t_i == 0),
                                             stop=(t_i == NPAIR - 2))
                            yield
                        nc.vector.tensor_copy(opart[2 * i + ng][:], ps[:])

            def oproj_final_unit(gq):
                # fine-grained: normalize one 128-col chunk (A on vector,
                # B on gpsimd, concurrent), then immediately project it, so
                # the tail chain is ~chunk-latency instead of full-gq norm
                t3 = NPAIR - 1
                ev = evs[t3]
                for i, m in enumerate(range(4 * gq, 4 * (gq + 1))):
                    cs = slice(P * m, P * (m + 1))
                    a0 = 1024 * gq + P * i
                    b0 = 1024 * gq + 512 + P * i
                    nc.vector.tensor_mul(conc[t3][0:64, cs],
                                         ev[0:64, a0:a0 + P], rcA[:, cs])
                    nc.gpsimd.tensor_mul(conc[t3][64:128, cs],
                                         ev[0:64, b0:b0 + P], rcB[:, cs])
                    for ng in range(2):
                        ps = alloc_ps()
                        nc.tensor.matmul(ps[:],
                                         conc[t3][:, cs],
                                         wo[t3][:, 512 * ng:512 * (ng + 1)],
                                         start=True, stop=True)
                        yield
                        osb = wkp.tile([P, 512], dt.bfloat16, tag="osb")
                        nc.vector.tensor_add(osb[:], ps[:],
                                             opart[2 * i + ng][:])
                        dma_store(out_d.ap()[P * m:P * (m + 1),
                                             512 * ng:512 * (ng + 1)], osb[:])

            # filler queue entries: (need_key, generator). need_key = (t, gq)
            # means the unit MUST be fully emitted before attention pair t's
            # seq-group gq emits its first score matmul (else the in-order PE
            # queue deadlocks on a score whose qrot/krot/vil producers sit
            # behind it). Pushes happen in need order, so FIFO = need order.
            fq = collections.deque()

            def pump(n=1, cap=None):
                # run up to n work-yields from the first queue entries whose
                # key is <= cap (skipping over-cap entries). The cap reserves
                # later pairs' projection units for their own attention
                # windows instead of letting early pairs strip-mine them.
                k = 0
                i = 0
                while k < n and i < len(fq):
                    key, gen = fq[i]
                    if cap is not None and key > cap:
                        i += 1
                        continue
                    try:
                        next(gen)
                        k += 1
                    except StopIteration:
                        del fq[i]

            def drain_until(key):
                i = 0
                while i < len(fq):
                    if fq[i][0] <= key:
                        gen = fq[i][1]
                        try:
                            while True:
                                next(gen)
                        except StopIteration:
                            del fq[i]
                    else:
                        i += 1

            def pump_all():
                drain_until((NPAIR + 2, 0))

            def attention(t_i, post_gq=None):
                last = (t_i == NPAIR - 1)
                cA, cB = VW * (2 * t_i), VW * (2 * t_i + 1)
                ev = evp.tile([65, 2 * S], dt.bfloat16, tag="ev",
                              name=f"ev{t_i}")
                evs[t_i] = ev
                for gq in range(NSG):
                    drain_until((t_i, gq))
                    nki = 4 * gq + 4
                    avAB = pup.tile([P, 1024], dt.float32, tag="av", bufs=1,
                                    name="avAB")
                    pend = {}

                    def emit_S(ki):
                        joff = max(0, P * ki - 512 * gq)
                        width = 512 - joff
                        qss = slice(512 * gq + joff, 512 * (gq + 1))
                        kss = slice(P * ki, P * (ki + 1))
                        sAB = pup.tile([P, 1024], dt.float32, tag="u", bufs=2,
                                       name="sAB")
                        nc.tensor.matmul(sAB[:, 0:width], krot[t_i][0:64, kss],
                                         qrot[t_i][0:64, qss],
                                         start=True, stop=True)
                        nc.tensor.matmul(sAB[:, 512:512 + width],
                                         krot[t_i][64:128, kss],
                                         qrot[t_i][64:128, qss],
                                         start=True, stop=True)
                        # exp split per half so AV_A only waits ~half the
                        # scalar latency; diag mask split likewise
                        ptAB = wkp.tile([P, 1024], dt.bfloat16, tag="pt")
                        diag = ki >= 4 * gq
                        # diag mask = multiply by a 0/1 triangle tile on the
                        # vector queue: gpsimd's in-order queue head-blocks
                        # the mask behind 1.5us norm fillers, stalling the AV
                        # matmuls that need it immediately
                        nc.scalar.activation(ptAB[:, 0:width], sAB[:, 0:width],
                                             EXP, bias=0.0, scale=0.125)
                        if diag:
                            nc.vector.tensor_mul(ptAB[:, 0:P],
                                                 ptAB[:, 0:P], tri[:])
                        nc.scalar.activation(ptAB[:, 512:512 + width],
                                             sAB[:, 512:512 + width],
                                             EXP, bias=0.0, scale=0.125)
                        if diag:
                            nc.vector.tensor_mul(ptAB[:, 512:512 + P],
                                                 ptAB[:, 512:512 + P], tri[:])
                        pend[ki] = (ptAB, joff, width)

                    def emit_AV_A(ki):
                        ptAB, joff, width = pend[ki]
                        nc.tensor.matmul(avAB[0:65, joff:512],
                                         vil[ki][:, cA:cA + 65],
                                         ptAB[:, 0:width],
                                         start=(ki == 0), stop=(ki == nki - 1))

                    def emit_AV_B(ki):
                        ptAB, joff, width = pend.pop(ki)
                        nc.tensor.matmul(avAB[0:65, 512 + joff:1024],
                                         vil[ki][:, cB:cB + 65],
                                         ptAB[:, 512:512 + width],
                                         start=(ki == 0), stop=(ki == nki - 1))

                    cap = (t_i + 1, NSG)
                    emit_S(0)
                    for ki in range(nki):
                        if ki + 1 < nki:
                            emit_S(ki + 1)
                        pump(1, cap)
                        emit_AV_A(ki)
                        pump(1, cap)
                        emit_AV_B(ki)
                        pump(2 if (last and gq == NSG - 1) else 1, cap)
                    # split drain copy so next gq's first AV only waits on
                    # its own half being freed
                    nc.vector.tensor_copy(ev[:, 1024 * gq:1024 * gq + 512],
                                          avAB[0:65, 0:512])
                    nc.vector.tensor_copy(ev[:, 1024 * gq + 512:1024 * (gq + 1)],
                                          avAB[0:65, 512:1024])
                    if post_gq is not None:
                        post_gq(gq)

            # ---- emission ------------------------------------------------
            # startup: round-robin 4 projection units at a time so the PE
            # instruction order matches the DMA chunk-arrival order (each
            # arriving (w,xt) chunk pair unlocks one matmul in each unit;
            # serial emission would head-block the in-order PE queue).
            def rr_drain(units):
                units = list(units)
                while units:
                    for g in list(units):
                        try:
                            next(g)
                        except StopIteration:
                            units.remove(g)

            rr_drain([qk_unit(0, 0, 0), qk_unit(1, 0, 0),
                      qk_unit(0, 0, 1), qk_unit(1, 0, 1)])
            rr_drain([v_unit(0), v_unit(1), v_unit(2), v_unit(3)])

            for sg in range(1, NSG):
                for which in (0, 1):
                    fq.append(((0, sg), qk_unit(0, sg, which)))
                for m in range(4 * sg, 4 * sg + 4):
                    fq.append(((0, sg), v_unit(m)))

            for t_i in range(NPAIR):
                if t_i + 1 < NPAIR:
                    for sg in range(NSG):
                        for which in (0, 1):
                            if t_i + 1 == 1 and sg == 0:
                                continue  # already emitted in startup RR
                            fq.append(((t_i + 1, sg),
                                       qk_unit(t_i + 1, sg, which)))
                last = (t_i == NPAIR - 1)

                def post_gq(gq, t_i=t_i, last=last):
                    fq.append(((t_i + 1, NSG), bc_unit(t_i, gq)))
                    if not (last and gq == NSG - 1):
                        fq.append(((t_i + 1, NSG), norm_unit(t_i, gq)))
                    if t_i == NPAIR - 2 and gq == NSG - 1:
                        fq.append(((t_i + 2, NSG), oproj_partial_unit(gq)))
                    if last:
                        if gq == NSG - 1:
                            fq.append(((t_i + 1, NSG), oproj_final_unit(gq)))
                        else:
                            fq.append(((t_i + 1, NSG), oproj_unit(gq)))

                attention(t_i, post_gq)
            pump_all()

    nc.compile()
    return nc


def _dim_perm():
    # on-chip head-dim order: rotate-half within each 32-partition block
    # (x1 of pairs 16b+0..16b+15 at block positions 0-15, x2 at 16-31), so
    # the rope pair swap is a stream_shuffle 32-permutation.
    p = np.arange(64)
    perm64 = 2 * (16 * (p // 32) + (p % 16)) + (p % 32) // 16
    return np.concatenate([64 * h + perm64 for h in range(8)])   # [512]


def _host_tables(token_positions):
    pos = np.asarray(token_positions).astype(np.float32)
    inv_freq = (THETA ** (-(np.arange(0, DK, 2, dtype=np.float32)) / DK))  # [32]
    ang = pos[:, None] * inv_freq[None, :]                                 # [s, 32]
    cos_t = np.cos(ang).T                                                  # [32, s]
    sin_t = np.sin(ang).T
    pp = np.arange(P)
    j = 16 * ((pp % 64) // 32) + (pp % 32) % 16   # freq index per partition
    sign = np.where((pp % 32) < 16, -1.0, 1.0).astype(np.float32)
    cosE = np.ascontiguousarray(cos_t[j, :]).astype(bf16)                  # [128, s]
    sinE = np.ascontiguousarray(sin_t[j, :] * sign[:, None]).astype(bf16)

    bcsel = np.zeros((P, 256), dtype=np.float32)
    bcsel[:, 0:64] = 1.0
    bcsel[:, 192:256] = 1.0
    bcsel = bcsel.astype(bf16)

    tri = (np.arange(P)[None, :] >= np.arange(P)[:, None]).astype(bf16)
    return cosE, sinE, bcsel, tri


def _in_maps(x, Wq, Wk, Wv, Wo, token_positions):
    cosE, sinE, bcsel, tri = _host_tables(token_positions)
    perm = _dim_perm()
    in_maps = []
    for c in range(8):
        b, g = c // 2, c % 2
        rows = slice(DH * g, DH * (g + 1))
        in_maps.append({
            "xt": np.ascontiguousarray(x[b].T).astype(bf16),
            "wq": np.ascontiguousarray(Wq[rows, :][perm, :].T).astype(bf16),
            "wk": np.ascontiguousarray(Wk[rows, :][perm, :].T).astype(bf16),
            "wv": np.ascontiguousarray(Wv[rows, :].T).astype(bf16),
            "wo": np.ascontiguousarray(Wo[:, rows].T).astype(bf16),
            "cosE": cosE, "sinE": sinE, "bcsel": bcsel,
            "tri": tri,
        })
    return in_maps


def kernel(in_features, Wq, Wk, Wv, Wo, token_positions):
    from concourse import bass_utils

    x = np.asarray(in_features, dtype=np.float32)
    Wq = np.asarray(Wq, dtype=np.float32)
    Wk = np.asarray(Wk, dtype=np.float32)
    Wv = np.asarray(Wv, dtype=np.float32)
    Wo = np.asarray(Wo, dtype=np.float32)

    if "nc" not in _CACHE:
        _CACHE["nc"] = _build_program()
    nc = _CACHE["nc"]

    in_maps = _in_maps(x, Wq, Wk, Wv, Wo, token_positions)
    res = bass_utils.run_bass_kernel_spmd(nc, in_maps, core_ids=list(range(8)))
    out = np.empty((B, S, D), dtype=np.float32)
    for b in range(B):
        out[b] = (res.results[2 * b]["out"].astype(np.float32)
                  + res.results[2 * b + 1]["out"].astype(np.float32))
    return out


# revision 50
# speedup vs baseline: 1.0684x; 1.0684x over previous
"""Causal multi-head self-attention with RoPE on 8 TRN2 NeuronCores.

Problem: b=4, s=2048, d_model=1024, 16 heads, d_k=64, fp32 I/O.

Sharding: core c = (batch b = c//2, head-half g = c%2). Each core computes the
8 heads of one head-half for one batch element, applies its slice of the
output projection, and returns a partial [2048, 1024] (bf16); the host sums
the two partials per batch in fp32 (the tensor-parallel all-reduce on host).

v3 schedule (v2 + pipeline/latency work):
- exp is split into per-head-half activations (and per-half diag masks +
  per-half PSUM drain copies) so each AV matmul waits ~600ns of scalar
  latency instead of ~1us, cutting per-step PE bubbles.
- the output projection is emitted per seq-group as filler work inside the
  LAST pair's attention stream (gated on that pair's normalization), so the
  former ~36us serial o_proj tail shrinks to one seq-group (~10us).
- pair-0 Q/K/V projection units go through the need-key queue like everyone
  else, so attention(0, gq0) starts after 2 qk units + 4 v units instead of
  after all 16; the input DMA order matches this consumption order.
- the two denominator-broadcast matmuls are column-tiled to disjoint output
  partitions (A->0:64, B->64:128) and run concurrently in the PE array, with
  one merged reciprocal.

Score matmul halves (contraction 64, partitions 0-63/64-127) auto-derive
tile_position (0,0)/(64,0) and stream concurrently in the PE array.

PSUM banking (8 banks of 2KB/partition): avAB accumulator tag "av" 1x2 banks,
score tiles tag "u" 2x2 banks (pipeline depth 2), projection tiles tag "pj"
2x1 banks.
"""

import collections
import numpy as np
import ml_dtypes

bf16 = ml_dtypes.bfloat16

N_HEADS = 16
THETA = 10000.0
B, S, D = 4, 2048, 1024
DK = D // N_HEADS          # 64
DH = D // 2                # 512 dims per core (8 heads)
P = 128
NKC = D // P               # 8 contraction chunks for projections
NSG = S // 512             # 4 seq groups of 512
NST = S // P               # 16 seq tiles of 128
NPAIR = DH // P            # 4 head-pair tiles per core
VW = 66                    # per-head stride in interleaved V tile

_CACHE = {}


def _build_program():
    import concourse.tile as tile
    from concourse import bacc, mybir

    nc = bacc.Bacc("TRN2", target_bir_lowering=False, debug=False, num_devices=1)
    dt = mybir.dt

    xt_d = nc.dram_tensor("xt", [D, S], dt.bfloat16, kind="ExternalInput")
    wq_d = nc.dram_tensor("wq", [D, DH], dt.bfloat16, kind="ExternalInput")
    wk_d = nc.dram_tensor("wk", [D, DH], dt.bfloat16, kind="ExternalInput")
    wv_d = nc.dram_tensor("wv", [D, DH], dt.bfloat16, kind="ExternalInput")
    wo_d = nc.dram_tensor("wo", [DH, D], dt.bfloat16, kind="ExternalInput")
    cos_d = nc.dram_tensor("cosE", [P, S], dt.bfloat16, kind="ExternalInput")
    sin_d = nc.dram_tensor("sinE", [P, S], dt.bfloat16, kind="ExternalInput")
    tri_d = nc.dram_tensor("tri", [P, P], dt.bfloat16, kind="ExternalInput")
    bc_d = nc.dram_tensor("bcsel", [P, 256], dt.bfloat16, kind="ExternalInput")
    out_d = nc.dram_tensor("out", [S, D], dt.bfloat16, kind="ExternalOutput")

    EXP = mybir.ActivationFunctionType.Exp

    with tile.TileContext(nc) as tc:
        with tc.tile_pool(name="const", bufs=1) as cst, \
             tc.tile_pool(name="persist", bufs=1) as per, \
             tc.tile_pool(name="work", bufs=4) as wkp, \
             tc.tile_pool(name="ev", bufs=2) as evp, \
             tc.tile_pool(name="up", bufs=2, space="PSUM") as pup:

            # ---- input DMA: rotate across 3 engine queues, ordered to match
            # compute consumption: (wq,xt_sg0,wk) + p2/cos/sin sg0 first, then
            # wv (for v units 0-3), then the later seq groups, bcsel, wo last.
            qs_ = [nc.sync, nc.scalar, nc.gpsimd]
            dma_i = [0]

            def dma_load(dst, src):
                qs_[dma_i[0] % 3].dma_start(dst, src)
                dma_i[0] += 1

            wq, wk_, wv, wo = [], [], [], []
            xt = [[None] * NSG for _ in range(NKC)]
            cosE = [None] * NSG
            sinE = [None] * NSG

            def _load_cs(sg):
                c = cst.tile([P, 512], dt.bfloat16, tag=f"cos{sg}", name=f"cos{sg}")
                dma_load(c[:], cos_d.ap()[:, 512 * sg:512 * (sg + 1)])
                cosE[sg] = c
                s = cst.tile([P, 512], dt.bfloat16, tag=f"sin{sg}", name=f"sin{sg}")
                dma_load(s[:], sin_d.ap()[:, 512 * sg:512 * (sg + 1)])
                sinE[sg] = s

            def _load_xt(kc, sg):
                t = cst.tile([P, 512], dt.bfloat16, tag=f"xt{kc}_{sg}",
                             name=f"xt{kc}_{sg}")
                dma_load(t[:], xt_d.ap()[P * kc:P * (kc + 1),
                                         512 * sg:512 * (sg + 1)])
                xt[kc][sg] = t

            for kc in range(NKC):
                t = cst.tile([P, DH], dt.bfloat16, tag=f"wq{kc}", name=f"wq{kc}")
                dma_load(t[:], wq_d.ap()[P * kc:P * (kc + 1), :])
                wq.append(t)
                _load_xt(kc, 0)
                t = cst.tile([P, DH], dt.bfloat16, tag=f"wk{kc}", name=f"wk{kc}")
                dma_load(t[:], wk_d.ap()[P * kc:P * (kc + 1), :])
                wk_.append(t)
                if kc == 1:
                    tri = cst.tile([P, P], dt.bfloat16, tag="tri")
                    dma_load(tri[:], tri_d.ap())
                if kc == 3:
                    _load_cs(0)
            for kc in range(NKC):
                t = cst.tile([P, DH], dt.bfloat16, tag=f"wv{kc}", name=f"wv{kc}")
                dma_load(t[:], wv_d.ap()[P * kc:P * (kc + 1), :])
                wv.append(t)
                _load_xt(kc, 1)
                if kc == 0:
                    _load_cs(1)
            bcsel = cst.tile([P, 256], dt.bfloat16, tag="bc")
            for kc in range(NKC):
                _load_xt(kc, 2)
                if kc == 0:
                    _load_cs(2)
                if kc == 4:
                    dma_load(bcsel[:], bc_d.ap())
            for kc in range(NKC):
                _load_xt(kc, 3)
                if kc == 0:
                    _load_cs(3)
            for kc in range(NPAIR):
                t = cst.tile([P, D], dt.bfloat16, tag=f"wo{kc}", name=f"wo{kc}")
                dma_load(t[:], wo_d.ap()[P * kc:P * (kc + 1), :])
                wo.append(t)

            qrot = [per.tile([P, S], dt.bfloat16, tag=f"qrot{t_i}",
                             name=f"qrot{t_i}") for t_i in range(NPAIR)]
            krot = [per.tile([P, S], dt.bfloat16, tag=f"krot{t_i}",
                             name=f"krot{t_i}") for t_i in range(NPAIR)]
            vil = [per.tile([P, 8 * VW], dt.bfloat16, tag=f"v{m}",
                            name=f"vil{m}") for m in range(NST)]
            conc = [per.tile([P, S], dt.bfloat16, tag=f"conc{t_i}",
                             name=f"conc{t_i}") for t_i in range(NPAIR)]
            rcA = evp.tile([64, S], dt.float32, tag="rcA", name="rcA", bufs=1)
            rcB = evp.tile([64, S], dt.float32, tag="rcB", name="rcB", bufs=1)
            evs = [None] * NPAIR

            # ---- output DMA: keep the busy scalar queue (exp) clear; split
            # each [128,512] store in half across distinct queues so the
            # final store drains ~2x faster (per-queue DMA bw is the limit).
            st_i = [0]

            def dma_store(dst, src):
                qs = (nc.sync, nc.scalar, nc.gpsimd)
                qs[st_i[0] % 3].dma_start(dst[:, 0:256], src[:, 0:256])
                qs[(st_i[0] + 1) % 3].dma_start(dst[:, 256:512], src[:, 256:512])
                st_i[0] += 2

            # ---- work-unit generators; each yield = one PE matmul emitted
            # "u" borrows the (idle-at-startup) score-tile banks so four
            # projection accumulators can be live at once during the
            # DMA-paced startup round-robin
            def alloc_ps(ptag="pj"):
                if ptag == "u":
                    t = pup.tile([P, 1024], dt.float32, tag="u", bufs=2,
                                 name="sAB")
                    return t[:, 0:512]
                return pup.tile([P, 512], dt.float32, tag="pj", bufs=2,
                                name="pjps")[:]

            # rope pair partner: dims are host-relabeled rotate-half-in-32
            # (x1 at block pos 0-15, x2 at 16-31) so the pair swap is a DVE
            # stream_shuffle instead of a PE permutation matmul; the sign
            # (-x2, +x1) is folded into the host sinE table.
            SWAP_MASK = list(range(16, 32)) + list(range(16))

            def qk_unit(t_i, sg, which, ptag="pj"):
                w_tiles, rot = ((wq, qrot), (wk_, krot))[which]
                ps = alloc_ps(ptag)
                for kc in range(NKC):
                    nc.tensor.matmul(ps, w_tiles[kc][:, P * t_i:P * (t_i + 1)],
                                     xt[kc][sg][:],
                                     start=(kc == 0), stop=(kc == NKC - 1))
                    yield
                qsb = wkp.tile([P, 512], dt.bfloat16, tag="qsb")
                nc.vector.tensor_copy(qsb[:], ps)
                gs = slice(512 * sg, 512 * (sg + 1))
                tmp1 = wkp.tile([P, 512], dt.bfloat16, tag="tmp1")
                nc.vector.tensor_mul(tmp1[:], qsb[:], cosE[sg][:])
                q2sb = wkp.tile([P, 512], dt.bfloat16, tag="q2sb")
                nc.vector.stream_shuffle(q2sb[:], qsb[:], SWAP_MASK)
                tmp2 = wkp.tile([P, 512], dt.bfloat16, tag="tmp2")
                nc.vector.tensor_mul(tmp2[:], q2sb[:], sinE[sg][:])
                nc.vector.tensor_add(rot[t_i][:, gs], tmp1[:], tmp2[:])

            def v_unit(m, ptag="pj"):
                ps = alloc_ps(ptag)
                sg, mo = divmod(m, 4)
                for kc in range(NKC):
                    nc.tensor.matmul(ps, xt[kc][sg][:, P * mo:P * (mo + 1)],
                                     wv[kc][:, :],
                                     start=(kc == 0), stop=(kc == NKC - 1))
                    yield
                v3 = vil[m][:].rearrange("p (h c) -> p h c", c=VW)
                nc.gpsimd.memset(v3[:, :, 64:65], 1.0)
                nc.vector.tensor_copy(v3[:, :, 0:64],
                                      ps.rearrange("p (h c) -> p h c", c=64))

            def bc_unit(t_i, gq):
                # broadcast the denominator row to 64 partitions + reciprocal
                ev = evs[t_i]
                gs = slice(512 * gq, 512 * (gq + 1))
                bcA = alloc_ps()
                nc.tensor.matmul(bcA[0:64, :], bcsel[64:65, 0:64],
                                 ev[64:65, 1024 * gq:1024 * gq + 512],
                                 start=True, stop=True)
                yield
                nc.vector.reciprocal_approx_fast(rcA[:, gs], bcA[0:64, :])
                bcB = alloc_ps()
                nc.tensor.matmul(bcB[0:64, :], bcsel[64:65, 0:64],
                                 ev[64:65, 1024 * gq + 512:1024 * (gq + 1)],
                                 start=True, stop=True)
                yield
                nc.vector.reciprocal_approx_fast(rcB[:, gs], bcB[0:64, :])

            def norm_unit(t_i, gq):
                ev = evs[t_i]
                gs = slice(512 * gq, 512 * (gq + 1))
                nc.gpsimd.tensor_mul(conc[t_i][0:64, gs],
                                     ev[0:64, 1024 * gq:1024 * gq + 512],
                                     rcA[:, gs])
                yield
                nc.gpsimd.tensor_mul(conc[t_i][64:128, gs],
                                     ev[0:64, 1024 * gq + 512:1024 * (gq + 1)],
                                     rcB[:, gs])
                yield

            def oproj_unit(gq):
                # output projection for this seq-group's 4 m-tiles; emitted
                # as filler inside the last pair's attention stream
                for m in range(4 * gq, 4 * (gq + 1)):
                    for ng in range(2):
                        ps = alloc_ps()
                        for t_i in range(NPAIR):
                            nc.tensor.matmul(ps[:],
                                             conc[t_i][:, P * m:P * (m + 1)],
                                             wo[t_i][:, 512 * ng:512 * (ng + 1)],
                                             start=(t_i == 0),
                                             stop=(t_i == NPAIR - 1))
                            yield
                        osb = wkp.tile([P, 512], dt.bfloat16, tag="osb")
                        nc.vector.tensor_copy(osb[:], ps[:])
                        dma_store(out_d.ap()[P * m:P * (m + 1),
                                             512 * ng:512 * (ng + 1)], osb[:])

            # the LAST seq-group of the last pair would otherwise leave 32
            # serial matmuls after the final normalization; precompute the
            # pairs-0..2 partial sums as fillers, leaving an 8-matmul tail.
            opart = [per.tile([P, 512], dt.bfloat16, tag=f"opart{i}",
                              name=f"opart{i}") for i in range(8)]

            def oproj_partial_unit(gq):
                for i, m in enumerate(range(4 * gq, 4 * (gq + 1))):
                    for ng in range(2):
                        ps = alloc_ps()
                        for t_i in range(NPAIR - 1):
                            nc.tensor.matmul(ps[:],
                                             conc[t_i][:, P * m:P * (m + 1)],
                                             wo[t_i][:, 512 * ng:512 * (ng + 1)],
                                             start=(t_i == 0),
                                             stop=(t_i == NPAIR - 2))
                            yield
                        nc.vector.tensor_copy(opart[2 * i + ng][:], ps[:])

            def oproj_final_unit(gq):
                # fine-grained: normalize one 128-col chunk (A on vector,
                # B on gpsimd, concurrent), then immediately project it, so
                # the tail chain is ~chunk-latency instead of full-gq norm
                t3 = NPAIR - 1
                ev = evs[t3]
                for i, m in enumerate(range(4 * gq, 4 * (gq + 1))):
                    cs = slice(P * m, P * (m + 1))
                    a0 = 1024 * gq + P * i
                    b0 = 1024 * gq + 512 + P * i
                    nc.vector.tensor_mul(conc[t3][0:64, cs],
                                         ev[0:64, a0:a0 + P], rcA[:, cs])
                    nc.gpsimd.tensor_mul(conc[t3][64:128, cs],
                                         ev[0:64, b0:b0 + P], rcB[:, cs])
                    for ng in range(2):
                        ps = alloc_ps()
                        nc.tensor.matmul(ps[:],
                                         conc[t3][:, cs],
                                         wo[t3][:, 512 * ng:512 * (ng + 1)],
                                         start=True, stop=True)
                        yield
                        osb = wkp.tile([P, 512], dt.bfloat16, tag="osb")
                        nc.vector.tensor_add(osb[:], ps[:],
                                             opart[2 * i + ng][:])
                        dma_store(out_d.ap()[P * m:P * (m + 1),
                                             512 * ng:512 * (ng + 1)], osb[:])

            # filler queue entries: (need_key, generator). need_key = (t, gq)
            # means the unit MUST be fully emitted before attention pair t's
            # seq-group gq emits its first score matmul (else the in-order PE
            # queue deadlocks on a score whose qrot/krot/vil producers sit
            # behind it). Pushes happen in need order, so FIFO = need order.
            fq = collections.deque()

            def pump(n=1, cap=None):
                # run up to n work-yields from the first queue entries whose
                # key is <= cap (skipping over-cap entries). The cap reserves
                # later pairs' projection units for their own attention
                # windows instead of letting early pairs strip-mine them.
                k = 0
                i = 0
                while k < n and i < len(fq):
                    key, gen = fq[i]
                    if cap is not None and key > cap:
                        i += 1
                        continue
                    try:
                        next(gen)
                        k += 1
                    except StopIteration:
                        del fq[i]

            def drain_until(key):
                i = 0
                while i < len(fq):
                    if fq[i][0] <= key:
                        gen = fq[i][1]
                        try:
                            while True:
                                next(gen)
                        except StopIteration:
                            del fq[i]
                    else:
                        i += 1

            def pump_all():
                drain_until((NPAIR + 2, 0))

            def attention(t_i, post_gq=None):
                last = (t_i == NPAIR - 1)
                cA, cB = VW * (2 * t_i), VW * (2 * t_i + 1)
                ev = evp.tile([65, 2 * S], dt.bfloat16, tag="ev",
                              name=f"ev{t_i}")
                evs[t_i] = ev
                for gq in range(NSG):
                    drain_until((t_i, gq))
                    nki = 4 * gq + 4
                    avAB = pup.tile([P, 1024], dt.float32, tag="av", bufs=1,
                                    name="avAB")
                    pend = {}

                    def emit_S(ki):
                        joff = max(0, P * ki - 512 * gq)
                        width = 512 - joff
                        qss = slice(512 * gq + joff, 512 * (gq + 1))
                        kss = slice(P * ki, P * (ki + 1))
                        sAB = pup.tile([P, 1024], dt.float32, tag="u", bufs=2,
                                       name="sAB")
                        nc.tensor.matmul(sAB[:, 0:width], krot[t_i][0:64, kss],
                                         qrot[t_i][0:64, qss],
                                         start=True, stop=True)
                        nc.tensor.matmul(sAB[:, 512:512 + width],
                                         krot[t_i][64:128, kss],
                                         qrot[t_i][64:128, qss],
                                         start=True, stop=True)
                        # exp split per half so AV_A only waits ~half the
                        # scalar latency; diag mask split likewise
                        ptAB = wkp.tile([P, 1024], dt.bfloat16, tag="pt")
                        diag = ki >= 4 * gq
                        # diag mask = multiply by a 0/1 triangle tile on the
                        # vector queue: gpsimd's in-order queue head-blocks
                        # the mask behind 1.5us norm fillers, stalling the AV
                        # matmuls that need it immediately
                        nc.scalar.activation(ptAB[:, 0:width], sAB[:, 0:width],
                                             EXP, bias=0.0, scale=0.125)
                        if diag:
                            nc.vector.tensor_mul(ptAB[:, 0:P],
                                                 ptAB[:, 0:P], tri[:])
                        nc.scalar.activation(ptAB[:, 512:512 + width],
                                             sAB[:, 512:512 + width],
                                             EXP, bias=0.0, scale=0.125)
                        if diag:
                            nc.vector.tensor_mul(ptAB[:, 512:512 + P],
                                                 ptAB[:, 512:512 + P], tri[:])
                        pend[ki] = (ptAB, joff, width)

                    def emit_AV_A(ki):
                        ptAB, joff, width = pend[ki]
                        nc.tensor.matmul(avAB[0:65, joff:512],
                                         vil[ki][:, cA:cA + 65],
                                         ptAB[:, 0:width],
                                         start=(ki == 0), stop=(ki == nki - 1))

                    def emit_AV_B(ki):
                        ptAB, joff, width = pend.pop(ki)
                        nc.tensor.matmul(avAB[0:65, 512 + joff:1024],
                                         vil[ki][:, cB:cB + 65],
                                         ptAB[:, 512:512 + width],
                                         start=(ki == 0), stop=(ki == nki - 1))

                    cap = (t_i + 1, NSG)
                    emit_S(0)
                    for ki in range(nki):
                        if ki + 1 < nki:
                            emit_S(ki + 1)
                        pump(1, cap)
                        emit_AV_A(ki)
                        pump(1, cap)
                        emit_AV_B(ki)
                        pump(2 if (last and gq == NSG - 1) else 1, cap)
                    # split drain copy so next gq's first AV only waits on
                    # its own half being freed
                    nc.vector.tensor_copy(ev[:, 1024 * gq:1024 * gq + 512],
                                          avAB[0:65, 0:512])
                    nc.vector.tensor_copy(ev[:, 1024 * gq + 512:1024 * (gq + 1)],
                                          avAB[0:65, 512:1024])
                    if post_gq is not None:
                        post_gq(gq)

            # ---- emission ------------------------------------------------
            # startup: round-robin 4 projection units at a time so the PE
            # instruction order matches the DMA chunk-arrival order (each
            # arriving (w,xt) chunk pair unlocks one matmul in each unit;
            # serial emission would head-block the in-order PE queue).
            def rr_drain(units):
                units = list(units)
                while units:
                    for g in list(units):
                        try:
                            next(g)
                        except StopIteration:
                            units.remove(g)

            rr_drain([qk_unit(0, 0, 0, "pj"), qk_unit(1, 0, 0, "u"),
                      qk_unit(0, 0, 1, "pj"), qk_unit(1, 0, 1, "u")])
            rr_drain([v_unit(0, "pj"), v_unit(1, "u"),
                      v_unit(2, "pj"), v_unit(3, "u")])

            for sg in range(1, NSG):
                for which in (0, 1):
                    fq.append(((0, sg), qk_unit(0, sg, which)))
                for m in range(4 * sg, 4 * sg + 4):
                    fq.append(((0, sg), v_unit(m)))

            for t_i in range(NPAIR):
                if t_i + 1 < NPAIR:
                    for sg in range(NSG):
                        for which in (0, 1):
                            if t_i + 1 == 1 and sg == 0:
                                continue  # already emitted in startup RR
                            fq.append(((t_i + 1, sg),
                                       qk_unit(t_i + 1, sg, which)))
                last = (t_i == NPAIR - 1)

                def post_gq(gq, t_i=t_i, last=last):
                    fq.append(((t_i + 1, NSG), bc_unit(t_i, gq)))
                    if not (last and gq == NSG - 1):
                        fq.append(((t_i + 1, NSG), norm_unit(t_i, gq)))
                    if t_i == NPAIR - 2 and gq == NSG - 1:
                        fq.append(((t_i + 2, NSG), oproj_partial_unit(gq)))
                    if last:
                        if gq == NSG - 1:
                            fq.append(((t_i + 1, NSG), oproj_final_unit(gq)))
                        else:
                            fq.append(((t_i + 1, NSG), oproj_unit(gq)))

                attention(t_i, post_gq)
            pump_all()

    nc.compile()
    return nc


def _dim_perm():
    # on-chip head-dim order: rotate-half within each 32-partition block
    # (x1 of pairs 16b+0..16b+15 at block positions 0-15, x2 at 16-31), so
    # the rope pair swap is a stream_shuffle 32-permutation.
    p = np.arange(64)
    perm64 = 2 * (16 * (p // 32) + (p % 16)) + (p % 32) // 16
    return np.concatenate([64 * h + perm64 for h in range(8)])   # [512]


def _host_tables(token_positions):
    pos = np.asarray(token_positions).astype(np.float32)
    inv_freq = (THETA ** (-(np.arange(0, DK, 2, dtype=np.float32)) / DK))  # [32]
    ang = pos[:, None] * inv_freq[None, :]                                 # [s, 32]
    cos_t = np.cos(ang).T                                                  # [32, s]
    sin_t = np.sin(ang).T
    pp = np.arange(P)
    j = 16 * ((pp % 64) // 32) + (pp % 32) % 16   # freq index per partition
    sign = np.where((pp % 32) < 16, -1.0, 1.0).astype(np.float32)
    cosE = np.ascontiguousarray(cos_t[j, :]).astype(bf16)                  # [128, s]
    sinE = np.ascontiguousarray(sin_t[j, :] * sign[:, None]).astype(bf16)

    bcsel = np.zeros((P, 256), dtype=np.float32)
    bcsel[:, 0:64] = 1.0
    bcsel[:, 192:256] = 1.0
    bcsel = bcsel.astype(bf16)

    tri = (np.arange(P)[None, :] >= np.arange(P)[:, None]).astype(bf16)
    return cosE, sinE, bcsel, tri


def _in_maps(x, Wq, Wk, Wv, Wo, token_positions):
    cosE, sinE, bcsel, tri = _host_tables(token_positions)
    perm = _dim_perm()
    in_maps = []
    for c in range(8):
        b, g = c // 2, c % 2
        rows = slice(DH * g, DH * (g + 1))
        in_maps.append({
            "xt": np.ascontiguousarray(x[b].T).astype(bf16),
            "wq": np.ascontiguousarray(Wq[rows, :][perm, :].T).astype(bf16),
            "wk": np.ascontiguousarray(Wk[rows, :][perm, :].T).astype(bf16),
            "wv": np.ascontiguousarray(Wv[rows, :].T).astype(bf16),
            "wo": np.ascontiguousarray(Wo[:, rows].T).astype(bf16),
            "cosE": cosE, "sinE": sinE, "bcsel": bcsel,
            "tri": tri,
        })
    return in_maps


def kernel(in_features, Wq, Wk, Wv, Wo, token_positions):
    from concourse import bass_utils

    x = np.asarray(in_features, dtype=np.float32)
    Wq = np.asarray(Wq, dtype=np.float32)
    Wk = np.asarray(Wk, dtype=np.float32)
    Wv = np.asarray(Wv, dtype=np.float32)
    Wo = np.asarray(Wo, dtype=np.float32)

    if "nc" not in _CACHE:
        _CACHE["nc"] = _build_program()
    nc = _CACHE["nc"]

    in_maps = _in_maps(x, Wq, Wk, Wv, Wo, token_positions)
    res = bass_utils.run_bass_kernel_spmd(nc, in_maps, core_ids=list(range(8)))
    out = np.empty((B, S, D), dtype=np.float32)
    for b in range(B):
        out[b] = (res.results[2 * b]["out"].astype(np.float32)
                  + res.results[2 * b + 1]["out"].astype(np.float32))
    return out


# revision 51
# speedup vs baseline: 1.0776x; 1.0086x over previous
"""Causal multi-head self-attention with RoPE on 8 TRN2 NeuronCores.

Problem: b=4, s=2048, d_model=1024, 16 heads, d_k=64, fp32 I/O.

Sharding: core c = (batch b = c//2, head-half g = c%2). Each core computes the
8 heads of one head-half for one batch element, applies its slice of the
output projection, and returns a partial [2048, 1024] (bf16); the host sums
the two partials per batch in fp32 (the tensor-parallel all-reduce on host).

Schedule (evolved from the v2 software-pipelined design):
- Score matmul halves (contraction 64, krot/qrot partitions 0-63 / 64-127)
  auto-derive tile_position (0,0)/(64,0) and stream CONCURRENTLY in the PE
  array (row tiling), so a score pair costs ~512 cycles, not 1024.
- RoPE pair swap is a DVE stream_shuffle: head dims are host-relabeled to
  rotate-half-within-32 (x1 at block pos 0-15, x2 at 16-31, matching
  stream_shuffle's per-32-block granularity) and the -x2/+x1 signs are
  folded into the host sinE table. No PE permutation matmul.
- exp is split into per-head-half activations so each AV matmul waits ~600ns
  of scalar latency instead of ~1us; the causal diag mask is a 0/1 triangle
  tensor_mul on the VECTOR queue (gpsimd head-blocks it behind norm fillers).
- startup round-robins 4 projection units (2 PSUM tags) so the PE
  instruction order matches the DMA chunk-arrival order; input DMA is
  ordered (wq,xt0,wk) -> wv -> later seq groups -> wo.
- the output projection is emitted per seq-group as filler inside the LAST
  pair's attention; for the final seq-group only pair-3's 1-matmul pass
  (+ SBUF partials for pairs 0-2, computed earlier as fillers) remains after
  the last normalization, with fine-grained per-m-tile norm chunks.
- filler pump is key-capped per pair so later pairs' projection units are
  reserved for their own attention windows.

PSUM banking (8 banks of 2KB/partition): avAB accumulator tag "av" 1x2 banks,
score tiles tag "u" 2x2 banks (pipeline depth 2), projection tiles tag "pj"
2x1 banks.
"""

import collections
import numpy as np
import ml_dtypes

bf16 = ml_dtypes.bfloat16

N_HEADS = 16
THETA = 10000.0
B, S, D = 4, 2048, 1024
DK = D // N_HEADS          # 64
DH = D // 2                # 512 dims per core (8 heads)
P = 128
NKC = D // P               # 8 contraction chunks for projections
NSG = S // 512             # 4 seq groups of 512
NST = S // P               # 16 seq tiles of 128
NPAIR = DH // P            # 4 head-pair tiles per core
VW = 66                    # per-head stride in interleaved V tile

_CACHE = {}


def _build_program():
    import concourse.tile as tile
    from concourse import bacc, mybir

    nc = bacc.Bacc("TRN2", target_bir_lowering=False, debug=False, num_devices=1)
    dt = mybir.dt

    xt_d = nc.dram_tensor("xt", [D, S], dt.bfloat16, kind="ExternalInput")
    wq_d = nc.dram_tensor("wq", [D, DH], dt.bfloat16, kind="ExternalInput")
    wk_d = nc.dram_tensor("wk", [D, DH], dt.bfloat16, kind="ExternalInput")
    wv_d = nc.dram_tensor("wv", [D, DH], dt.bfloat16, kind="ExternalInput")
    wo_d = nc.dram_tensor("wo", [DH, D], dt.bfloat16, kind="ExternalInput")
    cos_d = nc.dram_tensor("cosE", [P, S], dt.bfloat16, kind="ExternalInput")
    sin_d = nc.dram_tensor("sinE", [P, S], dt.bfloat16, kind="ExternalInput")
    tri_d = nc.dram_tensor("tri", [P, P], dt.bfloat16, kind="ExternalInput")
    bc_d = nc.dram_tensor("bcsel", [P, 256], dt.bfloat16, kind="ExternalInput")
    out_d = nc.dram_tensor("out", [S, D], dt.bfloat16, kind="ExternalOutput")

    EXP = mybir.ActivationFunctionType.Exp

    with tile.TileContext(nc) as tc:
        with tc.tile_pool(name="const", bufs=1) as cst, \
             tc.tile_pool(name="persist", bufs=1) as per, \
             tc.tile_pool(name="work", bufs=4) as wkp, \
             tc.tile_pool(name="ev", bufs=2) as evp, \
             tc.tile_pool(name="up", bufs=2, space="PSUM") as pup:

            # ---- input DMA: rotate across 3 engine queues, ordered to match
            # compute consumption: (wq,xt_sg0,wk) + p2/cos/sin sg0 first, then
            # wv (for v units 0-3), then the later seq groups, bcsel, wo last.
            qs_ = [nc.sync, nc.scalar, nc.gpsimd]
            dma_i = [0]

            def dma_load(dst, src):
                qs_[dma_i[0] % 3].dma_start(dst, src)
                dma_i[0] += 1

            wq, wk_, wv, wo = [], [], [], []
            xt = [[None] * NSG for _ in range(NKC)]
            cosE = [None] * NSG
            sinE = [None] * NSG

            def _load_cs(sg):
                c = cst.tile([P, 512], dt.bfloat16, tag=f"cos{sg}", name=f"cos{sg}")
                dma_load(c[:], cos_d.ap()[:, 512 * sg:512 * (sg + 1)])
                cosE[sg] = c
                s = cst.tile([P, 512], dt.bfloat16, tag=f"sin{sg}", name=f"sin{sg}")
                dma_load(s[:], sin_d.ap()[:, 512 * sg:512 * (sg + 1)])
                sinE[sg] = s

            def _load_xt(kc, sg):
                t = cst.tile([P, 512], dt.bfloat16, tag=f"xt{kc}_{sg}",
                             name=f"xt{kc}_{sg}")
                dma_load(t[:], xt_d.ap()[P * kc:P * (kc + 1),
                                         512 * sg:512 * (sg + 1)])
                xt[kc][sg] = t

            for kc in range(NKC):
                t = cst.tile([P, DH], dt.bfloat16, tag=f"wq{kc}", name=f"wq{kc}")
                dma_load(t[:], wq_d.ap()[P * kc:P * (kc + 1), :])
                wq.append(t)
                _load_xt(kc, 0)
                t = cst.tile([P, DH], dt.bfloat16, tag=f"wk{kc}", name=f"wk{kc}")
                dma_load(t[:], wk_d.ap()[P * kc:P * (kc + 1), :])
                wk_.append(t)
                if kc == 1:
                    tri = cst.tile([P, P], dt.bfloat16, tag="tri")
                    dma_load(tri[:], tri_d.ap())
                if kc == 3:
                    _load_cs(0)
            for kc in range(NKC):
                t = cst.tile([P, DH], dt.bfloat16, tag=f"wv{kc}", name=f"wv{kc}")
                dma_load(t[:], wv_d.ap()[P * kc:P * (kc + 1), :])
                wv.append(t)
                _load_xt(kc, 1)
                if kc == 0:
                    _load_cs(1)
            bcsel = cst.tile([P, 256], dt.bfloat16, tag="bc")
            for kc in range(NKC):
                _load_xt(kc, 2)
                if kc == 0:
                    _load_cs(2)
                if kc == 4:
                    dma_load(bcsel[:], bc_d.ap())
            for kc in range(NKC):
                _load_xt(kc, 3)
                if kc == 0:
                    _load_cs(3)
            for kc in range(NPAIR):
                t = cst.tile([P, D], dt.bfloat16, tag=f"wo{kc}", name=f"wo{kc}")
                dma_load(t[:], wo_d.ap()[P * kc:P * (kc + 1), :])
                wo.append(t)

            qrot = [per.tile([P, S], dt.bfloat16, tag=f"qrot{t_i}",
                             name=f"qrot{t_i}") for t_i in range(NPAIR)]
            krot = [per.tile([P, S], dt.bfloat16, tag=f"krot{t_i}",
                             name=f"krot{t_i}") for t_i in range(NPAIR)]
            vil = [per.tile([P, 8 * VW], dt.bfloat16, tag=f"v{m}",
                            name=f"vil{m}") for m in range(NST)]
            conc = [per.tile([P, S], dt.bfloat16, tag=f"conc{t_i}",
                             name=f"conc{t_i}") for t_i in range(NPAIR)]
            rcA = evp.tile([64, S], dt.float32, tag="rcA", name="rcA", bufs=1)
            rcB = evp.tile([64, S], dt.float32, tag="rcB", name="rcB", bufs=1)
            evs = [None] * NPAIR

            # ---- output DMA: keep the busy scalar queue (exp) clear; split
            # each [128,512] store in half across distinct queues so the
            # final store drains ~2x faster (per-queue DMA bw is the limit).
            st_i = [0]

            def dma_store(dst, src):
                qs = (nc.sync, nc.scalar, nc.gpsimd)
                qs[st_i[0] % 3].dma_start(dst[:, 0:256], src[:, 0:256])
                qs[(st_i[0] + 1) % 3].dma_start(dst[:, 256:512], src[:, 256:512])
                st_i[0] += 2

            # ---- work-unit generators; each yield = one PE matmul emitted
            # "u" borrows the (idle-at-startup) score-tile banks so four
            # projection accumulators can be live at once during the
            # DMA-paced startup round-robin
            def alloc_ps(ptag="pj"):
                if ptag == "u":
                    t = pup.tile([P, 1024], dt.float32, tag="u", bufs=2,
                                 name="sAB")
                    return t[:, 0:512]
                return pup.tile([P, 512], dt.float32, tag="pj", bufs=2,
                                name="pjps")[:]

            # rope pair partner: dims are host-relabeled rotate-half-in-32
            # (x1 at block pos 0-15, x2 at 16-31) so the pair swap is a DVE
            # stream_shuffle instead of a PE permutation matmul; the sign
            # (-x2, +x1) is folded into the host sinE table.
            SWAP_MASK = list(range(16, 32)) + list(range(16))

            def qk_unit(t_i, sg, which, ptag="pj"):
                w_tiles, rot = ((wq, qrot), (wk_, krot))[which]
                ps = alloc_ps(ptag)
                for kc in range(NKC):
                    nc.tensor.matmul(ps, w_tiles[kc][:, P * t_i:P * (t_i + 1)],
                                     xt[kc][sg][:],
                                     start=(kc == 0), stop=(kc == NKC - 1))
                    yield
                qsb = wkp.tile([P, 512], dt.bfloat16, tag="qsb")
                nc.vector.tensor_copy(qsb[:], ps)
                gs = slice(512 * sg, 512 * (sg + 1))
                tmp1 = wkp.tile([P, 512], dt.bfloat16, tag="tmp1")
                nc.vector.tensor_mul(tmp1[:], qsb[:], cosE[sg][:])
                q2sb = wkp.tile([P, 512], dt.bfloat16, tag="q2sb")
                nc.vector.stream_shuffle(q2sb[:], qsb[:], SWAP_MASK)
                tmp2 = wkp.tile([P, 512], dt.bfloat16, tag="tmp2")
                nc.vector.tensor_mul(tmp2[:], q2sb[:], sinE[sg][:])
                nc.vector.tensor_add(rot[t_i][:, gs], tmp1[:], tmp2[:])

            def v_unit(m, ptag="pj"):
                ps = alloc_ps(ptag)
                sg, mo = divmod(m, 4)
                for kc in range(NKC):
                    nc.tensor.matmul(ps, xt[kc][sg][:, P * mo:P * (mo + 1)],
                                     wv[kc][:, :],
                                     start=(kc == 0), stop=(kc == NKC - 1))
                    yield
                v3 = vil[m][:].rearrange("p (h c) -> p h c", c=VW)
                nc.gpsimd.memset(v3[:, :, 64:65], 1.0)
                nc.vector.tensor_copy(v3[:, :, 0:64],
                                      ps.rearrange("p (h c) -> p h c", c=64))

            def bc_unit(t_i, gq):
                # broadcast the denominator row to 64 partitions + reciprocal
                ev = evs[t_i]
                gs = slice(512 * gq, 512 * (gq + 1))
                bcA = alloc_ps()
                nc.tensor.matmul(bcA[0:64, :], bcsel[64:65, 0:64],
                                 ev[64:65, 1024 * gq:1024 * gq + 512],
                                 start=True, stop=True)
                yield
                nc.vector.reciprocal_approx_fast(rcA[:, gs], bcA[0:64, :])
                bcB = alloc_ps()
                nc.tensor.matmul(bcB[0:64, :], bcsel[64:65, 0:64],
                                 ev[64:65, 1024 * gq + 512:1024 * (gq + 1)],
                                 start=True, stop=True)
                yield
                nc.vector.reciprocal_approx_fast(rcB[:, gs], bcB[0:64, :])

            def norm_unit(t_i, gq):
                ev = evs[t_i]
                gs = slice(512 * gq, 512 * (gq + 1))
                nc.gpsimd.tensor_mul(conc[t_i][0:64, gs],
                                     ev[0:64, 1024 * gq:1024 * gq + 512],
                                     rcA[:, gs])
                yield
                nc.gpsimd.tensor_mul(conc[t_i][64:128, gs],
                                     ev[0:64, 1024 * gq + 512:1024 * (gq + 1)],
                                     rcB[:, gs])
                yield

            def oproj_unit(gq):
                # output projection for this seq-group's 4 m-tiles; emitted
                # as filler inside the last pair's attention stream
                for m in range(4 * gq, 4 * (gq + 1)):
                    for ng in range(2):
                        ps = alloc_ps()
                        for t_i in range(NPAIR):
                            nc.tensor.matmul(ps[:],
                                             conc[t_i][:, P * m:P * (m + 1)],
                                             wo[t_i][:, 512 * ng:512 * (ng + 1)],
                                             start=(t_i == 0),
                                             stop=(t_i == NPAIR - 1))
                            yield
                        osb = wkp.tile([P, 512], dt.bfloat16, tag="osb")
                        nc.vector.tensor_copy(osb[:], ps[:])
                        dma_store(out_d.ap()[P * m:P * (m + 1),
                                             512 * ng:512 * (ng + 1)], osb[:])

            # the LAST seq-group of the last pair would otherwise leave 32
            # serial matmuls after the final normalization; precompute the
            # pairs-0..2 partial sums as fillers, leaving an 8-matmul tail.
            opart = [per.tile([P, 512], dt.bfloat16, tag=f"opart{i}",
                              name=f"opart{i}") for i in range(8)]

            def oproj_partial_unit(gq):
                for i, m in enumerate(range(4 * gq, 4 * (gq + 1))):
                    for ng in range(2):
                        ps = alloc_ps()
                        for t_i in range(NPAIR - 1):
                            nc.tensor.matmul(ps[:],
                                             conc[t_i][:, P * m:P * (m + 1)],
                                             wo[t_i][:, 512 * ng:512 * (ng + 1)],
                                             start=(t_i == 0),
                                             stop=(t_i == NPAIR - 2))
                            yield
                        nc.vector.tensor_copy(opart[2 * i + ng][:], ps[:])

            def oproj_final_unit(gq):
                # fine-grained: normalize one 128-col chunk (A on vector,
                # B on gpsimd, concurrent), then immediately project it, so
                # the tail chain is ~chunk-latency instead of full-gq norm
                t3 = NPAIR - 1
                ev = evs[t3]
                for i, m in enumerate(range(4 * gq, 4 * (gq + 1))):
                    cs = slice(P * m, P * (m + 1))
                    a0 = 1024 * gq + P * i
                    b0 = 1024 * gq + 512 + P * i
                    nc.vector.tensor_mul(conc[t3][0:64, cs],
                                         ev[0:64, a0:a0 + P], rcA[:, cs])
                    nc.gpsimd.tensor_mul(conc[t3][64:128, cs],
                                         ev[0:64, b0:b0 + P], rcB[:, cs])
                    for ng in range(2):
                        ps = alloc_ps()
                        nc.tensor.matmul(ps[:],
                                         conc[t3][:, cs],
                                         wo[t3][:, 512 * ng:512 * (ng + 1)],
                                         start=True, stop=True)
                        yield
                        osb = wkp.tile([P, 512], dt.bfloat16, tag="osb")
                        nc.vector.tensor_add(osb[:], ps[:],
                                             opart[2 * i + ng][:])
                        dma_store(out_d.ap()[P * m:P * (m + 1),
                                             512 * ng:512 * (ng + 1)], osb[:])

            # filler queue entries: (need_key, generator). need_key = (t, gq)
            # means the unit MUST be fully emitted before attention pair t's
            # seq-group gq emits its first score matmul (else the in-order PE
            # queue deadlocks on a score whose qrot/krot/vil producers sit
            # behind it). Pushes happen in need order, so FIFO = need order.
            fq = collections.deque()

            def pump(n=1, cap=None):
                # run up to n work-yields from the first queue entries whose
                # key is <= cap (skipping over-cap entries). The cap reserves
                # later pairs' projection units for their own attention
                # windows instead of letting early pairs strip-mine them.
                k = 0
                i = 0
                while k < n and i < len(fq):
                    key, gen = fq[i]
                    if cap is not None and key > cap:
                        i += 1
                        continue
                    try:
                        next(gen)
                        k += 1
                    except StopIteration:
                        del fq[i]

            def drain_until(key):
                i = 0
                while i < len(fq):
                    if fq[i][0] <= key:
                        gen = fq[i][1]
                        try:
                            while True:
                                next(gen)
                        except StopIteration:
                            del fq[i]
                    else:
                        i += 1

            def pump_all():
                drain_until((NPAIR + 2, 0))

            def attention(t_i, post_gq=None):
                last = (t_i == NPAIR - 1)
                cA, cB = VW * (2 * t_i), VW * (2 * t_i + 1)
                ev = evp.tile([65, 2 * S], dt.bfloat16, tag="ev",
                              name=f"ev{t_i}")
                evs[t_i] = ev
                for gq in range(NSG):
                    drain_until((t_i, gq))
                    nki = 4 * gq + 4
                    avAB = pup.tile([P, 1024], dt.float32, tag="av", bufs=1,
                                    name="avAB")
                    pend = {}

                    def emit_S(ki):
                        joff = max(0, P * ki - 512 * gq)
                        width = 512 - joff
                        qss = slice(512 * gq + joff, 512 * (gq + 1))
                        kss = slice(P * ki, P * (ki + 1))
                        sAB = pup.tile([P, 1024], dt.float32, tag="u", bufs=2,
                                       name="sAB")
                        nc.tensor.matmul(sAB[:, 0:width], krot[t_i][0:64, kss],
                                         qrot[t_i][0:64, qss],
                                         start=True, stop=True)
                        nc.tensor.matmul(sAB[:, 512:512 + width],
                                         krot[t_i][64:128, kss],
                                         qrot[t_i][64:128, qss],
                                         start=True, stop=True)
                        # exp split per half so AV_A only waits ~half the
                        # scalar latency; diag mask split likewise
                        ptAB = wkp.tile([P, 1024], dt.bfloat16, tag="pt")
                        diag = ki >= 4 * gq
                        # diag mask = multiply by a 0/1 triangle tile on the
                        # vector queue: gpsimd's in-order queue head-blocks
                        # the mask behind 1.5us norm fillers, stalling the AV
                        # matmuls that need it immediately
                        nc.scalar.activation(ptAB[:, 0:width], sAB[:, 0:width],
                                             EXP, bias=0.0, scale=0.125)
                        if diag:
                            nc.vector.tensor_mul(ptAB[:, 0:P],
                                                 ptAB[:, 0:P], tri[:])
                        nc.scalar.activation(ptAB[:, 512:512 + width],
                                             sAB[:, 512:512 + width],
                                             EXP, bias=0.0, scale=0.125)
                        if diag:
                            nc.vector.tensor_mul(ptAB[:, 512:512 + P],
                                                 ptAB[:, 512:512 + P], tri[:])
                        pend[ki] = (ptAB, joff, width)

                    def emit_AV_A(ki):
                        ptAB, joff, width = pend[ki]
                        nc.tensor.matmul(avAB[0:65, joff:512],
                                         vil[ki][:, cA:cA + 65],
                                         ptAB[:, 0:width],
                                         start=(ki == 0), stop=(ki == nki - 1))

                    def emit_AV_B(ki):
                        ptAB, joff, width = pend.pop(ki)
                        nc.tensor.matmul(avAB[0:65, 512 + joff:1024],
                                         vil[ki][:, cB:cB + 65],
                                         ptAB[:, 512:512 + width],
                                         start=(ki == 0), stop=(ki == nki - 1))

                    cap = (t_i + 1, NSG)
                    emit_S(0)
                    for ki in range(nki):
                        if ki + 1 < nki:
                            emit_S(ki + 1)
                        pump(1, cap)
                        emit_AV_A(ki)
                        pump(1, cap)
                        emit_AV_B(ki)
                        pump(2 if (last and gq == NSG - 1) else 1, cap)
                    # split drain copy so next gq's first AV only waits on
                    # its own half being freed
                    nc.vector.tensor_copy(ev[:, 1024 * gq:1024 * gq + 512],
                                          avAB[0:65, 0:512])
                    nc.vector.tensor_copy(ev[:, 1024 * gq + 512:1024 * (gq + 1)],
                                          avAB[0:65, 512:1024])
                    if post_gq is not None:
                        post_gq(gq)

            # ---- emission ------------------------------------------------
            # startup: round-robin 4 projection units at a time so the PE
            # instruction order matches the DMA chunk-arrival order (each
            # arriving (w,xt) chunk pair unlocks one matmul in each unit;
            # serial emission would head-block the in-order PE queue).
            def rr_drain(units):
                units = list(units)
                while units:
                    for g in list(units):
                        try:
                            next(g)
                        except StopIteration:
                            units.remove(g)

            rr_drain([qk_unit(0, 0, 0, "pj"), qk_unit(1, 0, 0, "u"),
                      qk_unit(0, 0, 1, "pj"), qk_unit(1, 0, 1, "u")])
            rr_drain([v_unit(0, "pj"), v_unit(1, "u"),
                      v_unit(2, "pj"), v_unit(3, "u")])

            for sg in range(1, NSG):
                for which in (0, 1):
                    fq.append(((0, sg), qk_unit(0, sg, which)))
                for m in range(4 * sg, 4 * sg + 4):
                    fq.append(((0, sg), v_unit(m)))

            for t_i in range(NPAIR):
                if t_i + 1 < NPAIR:
                    for sg in range(NSG):
                        for which in (0, 1):
                            if t_i + 1 == 1 and sg == 0:
                                continue  # already emitted in startup RR
                            fq.append(((t_i + 1, sg),
                                       qk_unit(t_i + 1, sg, which)))
                last = (t_i == NPAIR - 1)

                def post_gq(gq, t_i=t_i, last=last):
                    fq.append(((t_i + 1, NSG), bc_unit(t_i, gq)))
                    if not (last and gq == NSG - 1):
                        fq.append(((t_i + 1, NSG), norm_unit(t_i, gq)))
                    if t_i == NPAIR - 2 and gq == NSG - 1:
                        fq.append(((t_i + 2, NSG), oproj_partial_unit(gq)))
                    if last:
                        if gq == NSG - 1:
                            fq.append(((t_i + 1, NSG), oproj_final_unit(gq)))
                        else:
                            fq.append(((t_i + 1, NSG), oproj_unit(gq)))

                attention(t_i, post_gq)
            pump_all()

    nc.compile()
    return nc


def _dim_perm():
    # on-chip head-dim order: rotate-half within each 32-partition block
    # (x1 of pairs 16b+0..16b+15 at block positions 0-15, x2 at 16-31), so
    # the rope pair swap is a stream_shuffle 32-permutation.
    p = np.arange(64)
    perm64 = 2 * (16 * (p // 32) + (p % 16)) + (p % 32) // 16
    return np.concatenate([64 * h + perm64 for h in range(8)])   # [512]


def _host_tables(token_positions):
    pos = np.asarray(token_positions).astype(np.float32)
    inv_freq = (THETA ** (-(np.arange(0, DK, 2, dtype=np.float32)) / DK))  # [32]
    ang = pos[:, None] * inv_freq[None, :]                                 # [s, 32]
    cos_t = np.cos(ang).T                                                  # [32, s]
    sin_t = np.sin(ang).T
    pp = np.arange(P)
    j = 16 * ((pp % 64) // 32) + (pp % 32) % 16   # freq index per partition
    sign = np.where((pp % 32) < 16, -1.0, 1.0).astype(np.float32)
    cosE = np.ascontiguousarray(cos_t[j, :]).astype(bf16)                  # [128, s]
    sinE = np.ascontiguousarray(sin_t[j, :] * sign[:, None]).astype(bf16)

    bcsel = np.zeros((P, 256), dtype=np.float32)
    bcsel[:, 0:64] = 1.0
    bcsel[:, 192:256] = 1.0
    bcsel = bcsel.astype(bf16)

    tri = (np.arange(P)[None, :] >= np.arange(P)[:, None]).astype(bf16)
    return cosE, sinE, bcsel, tri


def _in_maps(x, Wq, Wk, Wv, Wo, token_positions):
    cosE, sinE, bcsel, tri = _host_tables(token_positions)
    perm = _dim_perm()
    in_maps = []
    for c in range(8):
        b, g = c // 2, c % 2
        rows = slice(DH * g, DH * (g + 1))
        in_maps.append({
            "xt": np.ascontiguousarray(x[b].T).astype(bf16),
            "wq": np.ascontiguousarray(Wq[rows, :][perm, :].T).astype(bf16),
            "wk": np.ascontiguousarray(Wk[rows, :][perm, :].T).astype(bf16),
            "wv": np.ascontiguousarray(Wv[rows, :].T).astype(bf16),
            "wo": np.ascontiguousarray(Wo[:, rows].T).astype(bf16),
            "cosE": cosE, "sinE": sinE, "bcsel": bcsel,
            "tri": tri,
        })
    return in_maps


def kernel(in_features, Wq, Wk, Wv, Wo, token_positions):
    from concourse import bass_utils

    x = np.asarray(in_features, dtype=np.float32)
    Wq = np.asarray(Wq, dtype=np.float32)
    Wk = np.asarray(Wk, dtype=np.float32)
    Wv = np.asarray(Wv, dtype=np.float32)
    Wo = np.asarray(Wo, dtype=np.float32)

    if "nc" not in _CACHE:
        _CACHE["nc"] = _build_program()
    nc = _CACHE["nc"]

    in_maps = _in_maps(x, Wq, Wk, Wv, Wo, token_positions)
    res = bass_utils.run_bass_kernel_spmd(nc, in_maps, core_ids=list(range(8)))
    out = np.empty((B, S, D), dtype=np.float32)
    for b in range(B):
        out[b] = (res.results[2 * b]["out"].astype(np.float32)
                  + res.results[2 * b + 1]["out"].astype(np.float32))
    return out
